# revision 1
# baseline (speedup 1.0000x reference)
"""ConvModLayer (StyleGAN2-style modulated 3x3 conv) on 8 Trainium2
NeuronCores — data-parallel over the batch (16 samples -> 2 per core).

Math (equivalent to the reference):
  cscale = 1/sqrt(512*9)
  s' = s * cscale
  sigma_sq[b,o] = sum_{i,ky,kx} (weight[o,i,ky,kx] * s'[b,i])^2
  out[b] = conv3x3(x[b] * s'[b,:,None,None], weight) * rsqrt(sigma_sq[b] + eps)

Device kernel (per core, identical SPMD program):
  - conv done as 9 shifted matmuls accumulated in PSUM over a
    zero-padded 66-wide image layout, operands in float32r
    (fp32 rounded to 11 mantissa bits) for full PE rate
  - sigma_sq via 144 tiny matmuls of squared weights against s'^2
  - PSUM -> SBUF copy fused with the rsqrt(sigma) channel scale

Host does only sharding/layout: batch slicing, weight transpose to
matmul layout (+ f32r pre-round, the device matmul input format), and
s reshape.
"""

import sys
from contextlib import ExitStack

if "/opt/trn_rl_repo" not in sys.path:
    sys.path.insert(0, "/opt/trn_rl_repo")

import numpy as np

import concourse.bacc as bacc
import concourse.mybir as mybir
import concourse.tile as tile
from concourse.bass_utils import run_bass_kernel_spmd

F32 = mybir.dt.float32
F32R = mybir.dt.float32r

N_CORES = 8
B = 16
B2 = B // N_CORES  # samples per core
C = 512
NCH = 4  # 128-partition channel chunks
H = W = 64
EPS = 1e-8
CSCALE = 1.0 / (C * 9) ** 0.5

_NC_CACHE = {}


def _build(psum_bufs: int = 6, yb_inner: bool = True, raw_bufs: int = 3):
    nc = bacc.Bacc("TRN2", target_bir_lowering=False, debug=False)

    x_d = nc.dram_tensor("x", [B2, C, H, W], F32, kind="ExternalInput")
    s_d = nc.dram_tensor("s", [128, NCH, B2], F32, kind="ExternalInput")
    w_d = nc.dram_tensor("w", [128, 9, NCH, C], F32R, kind="ExternalInput")
    o_d = nc.dram_tensor("o", [B2, C, H, W], F32, kind="ExternalOutput")

    with tile.TileContext(nc) as tc, ExitStack() as ctx:
        wpool = ctx.enter_context(tc.tile_pool(name="wpool", bufs=1))
        spool = ctx.enter_context(tc.tile_pool(name="spool", bufs=1))
        sqpool = ctx.enter_context(tc.tile_pool(name="sqpool", bufs=2))
        rawpool = ctx.enter_context(tc.tile_pool(name="rawpool", bufs=raw_bufs))
        xpool = ctx.enter_context(tc.tile_pool(name="xpool", bufs=2))
        opool = ctx.enter_context(tc.tile_pool(name="opool", bufs=4))
        pspool = ctx.enter_context(
            tc.tile_pool(name="pspool", bufs=psum_bufs, space="PSUM")
        )
        sigps = ctx.enter_context(tc.tile_pool(name="sigps", bufs=1, space="PSUM"))

        # ---- weights (pre-rounded f32r), split DMA so PE can start early ----
        w_r = wpool.tile([128, 9, NCH, C], F32R)
        for kpos in range(9):
            nc.sync.dma_start(w_r[:, kpos], w_d[:, kpos])
        w_f = w_r[:].bitcast(F32)  # rounded values, plain-f32 view

        s_t = spool.tile([128, NCH, B2], F32)
        nc.sync.dma_start(s_t[:], s_d[:])
        nc.vector.tensor_scalar_mul(s_t[:], s_t[:], CSCALE)
        s2_t = spool.tile([128, NCH, B2], F32)
        nc.vector.tensor_mul(s2_t[:], s_t[:], s_t[:])

        # zeros for pad regions (f32 source for convert-copies)
        z66 = spool.tile([128, 66], F32)
        nc.vector.tensor_scalar_mul(z66[:], w_f[:, 0, 0, 0:66], 0.0)

        # ---- sigma_sq[b, o] = sum_{i,k} w2[i,o] * s'2[i,b] ----
        psig = sigps.tile([128, NCH, B2], F32)
        for kpos in range(9):
            for ic in range(NCH):
                sq = sqpool.tile([128, C], F32)
                nc.vector.tensor_mul(sq[:], w_f[:, kpos, ic], w_f[:, kpos, ic])
                for oc in range(NCH):
                    # start=True clears the WHOLE bank -> only the global
                    # first matmul sets it; later groups overwrite-where-
                    # unset via per-element has_written bits.
                    nc.tensor.matmul(
                        psig[:, oc, :],
                        sq[:, oc * 128 : (oc + 1) * 128],
                        s2_t[:, ic, :],
                        start=(kpos == 0 and ic == 0 and oc == 0),
                        stop=(kpos == 8 and ic == 3 and oc == 3),
                        skip_group_check=True,
                    )
        sig_t = spool.tile([128, NCH, B2], F32)
        nc.vector.tensor_scalar_add(sig_t[:], psig[:], EPS)
        nc.scalar.sqrt(sig_t[:], sig_t[:])
        nc.vector.reciprocal(sig_t[:], sig_t[:])

        # ---- conv: per sample, 2 halves of 32 output rows ----
        # xt rows 0..33 = zero-padded image rows [h*32, h*32+34)
        for b in range(B2):
            for h in range(2):
                xts = []
                for ic in range(NCH):
                    raw = rawpool.tile([128, 33, 64], F32, tag="raw", name="raw")
                    r0 = 0 if h == 0 else 31
                    nc.sync.dma_start(
                        raw[:], x_d[b, ic * 128 : (ic + 1) * 128, r0 : r0 + 33, :]
                    )
                    xt = xpool.tile([128, 34, 66], F32R, tag=f"xt{ic}", name="xt")
                    nc.vector.tensor_copy(xt[:, :, 0], z66[:, 0:34])
                    nc.vector.tensor_copy(xt[:, :, 65], z66[:, 0:34])
                    if h == 0:
                        nc.vector.tensor_copy(xt[:, 0, :], z66[:, 0:66])
                        dst = xt[:, 1:34, 1:65]
                    else:
                        nc.vector.tensor_copy(xt[:, 33, :], z66[:, 0:66])
                        dst = xt[:, 0:33, 1:65]
                    # scale by s' and round to f32r
                    nc.vector.tensor_scalar_mul(dst, raw[:], s_t[:, ic, b : b + 1])
                    xts.append(xt)

                for oc in range(NCH):
                    if yb_inner:
                        accs = [
                            pspool.tile([128, 512], F32, tag="acc", name=f"acc{yy}")
                            for yy in range(4)
                        ]
                        for kpos in range(9):
                            ky, kx = divmod(kpos, 3)
                            for ic in range(NCH):
                                lhsT = w_r[:, kpos, ic, oc * 128 : (oc + 1) * 128]
                                for yb in range(4):
                                    nc.tensor.matmul(
                                        accs[yb][:],
                                        lhsT,
                                        xts[ic][
                                            :,
                                            yb * 8 + ky : yb * 8 + ky + 8,
                                            kx : kx + 64,
                                        ],
                                        start=(kpos == 0 and ic == 0),
                                        stop=(kpos == 8 and ic == 3),
                                    )
                    else:
                        accs = []
                        for yb in range(4):
                            acc = pspool.tile([128, 512], F32, tag="acc", name="acc")
                            accs.append(acc)
                            for kpos in range(9):
                                ky, kx = divmod(kpos, 3)
                                for ic in range(NCH):
                                    nc.tensor.matmul(
                                        acc[:],
                                        w_r[:, kpos, ic, oc * 128 : (oc + 1) * 128],
                                        xts[ic][
                                            :,
                                            yb * 8 + ky : yb * 8 + ky + 8,
                                            kx : kx + 64,
                                        ],
                                        start=(kpos == 0 and ic == 0),
                                        stop=(kpos == 8 and ic == 3),
                                    )
                    for yb in range(4):
                        out_t = opool.tile([128, 512], F32, tag="out", name="out")
                        nc.scalar.mul(out_t[:], accs[yb][:], sig_t[:, oc, b : b + 1])
                        y0 = h * 32 + yb * 8
                        nc.sync.dma_start(
                            o_d[b, oc * 128 : (oc + 1) * 128, y0 : y0 + 8, :],
                            out_t[:],
                        )

    nc.compile()
    return nc


def get_nc(**kwargs):
    key = tuple(sorted(kwargs.items()))
    if key not in _NC_CACHE:
        _NC_CACHE[key] = _build(**kwargs)
    return _NC_CACHE[key]


def _round_f32r(x: np.ndarray) -> np.ndarray:
    """Round fp32 to 11 mantissa bits (RNE) — the f32r matmul format."""
    u = np.ascontiguousarray(x).view(np.uint32)
    low = u & np.uint32(0xFFF)
    half = np.uint32(0x800)
    lsb = (u >> np.uint32(12)) & np.uint32(1)
    rnd = (low > half) | ((low == half) & (lsb == 1))
    out = (u & np.uint32(0xFFFFF000)) + (rnd.astype(np.uint32) << np.uint32(12))
    return out.view(np.float32)


def make_in_maps(x, s, weight):
    """Shard full inputs into 8 per-core input maps."""
    x = np.asarray(x, dtype=np.float32)
    s = np.asarray(s, dtype=np.float32)
    weight = np.asarray(weight, dtype=np.float32)
    w_prep = np.ascontiguousarray(
        weight.reshape(C, NCH, 128, 3, 3).transpose(2, 3, 4, 1, 0).reshape(
            128, 9, NCH, C
        )
    )
    w_prep = _round_f32r(w_prep)
    in_maps = []
    for core in range(N_CORES):
        xs = np.ascontiguousarray(x[core * B2 : (core + 1) * B2])
        ss = np.ascontiguousarray(
            s[core * B2 : (core + 1) * B2].reshape(B2, NCH, 128).transpose(2, 1, 0)
        )
        in_maps.append({"x": xs, "s": ss, "w": w_prep})
    return in_maps


def kernel(x, s, weight):
    nc = get_nc()
    in_maps = make_in_maps(x, s, weight)
    res = run_bass_kernel_spmd(nc, in_maps, list(range(N_CORES)))
    out = np.concatenate([r["o"] for r in res.results], axis=0)
    return out.astype(np.float32)


# revision 6
# speedup vs baseline: 1.0724x; 1.0724x over previous
"""ConvModLayer (StyleGAN2-style modulated 3x3 conv) on 8 Trainium2
NeuronCores — data-parallel over the batch (16 samples -> 2 per core).

Math (equivalent to the reference):
  cscale = 1/sqrt(512*9)
  s' = s * cscale
  sigma_sq[b,o] = sum_{i,ky,kx} (weight[o,i,ky,kx] * s'[b,i])^2
  out[b] = conv3x3(x[b] * s'[b,:,None,None], weight) * rsqrt(sigma_sq[b] + eps)

Device kernel (per core, identical SPMD program):
  - conv done as 9 shifted matmuls accumulated in PSUM over a
    zero-padded 66-wide image layout, operands in float32r
    (fp32 rounded to 11 mantissa bits) for full PE rate
  - sigma_sq via 144 tiny matmuls of squared weights against s'^2
  - PSUM -> SBUF copy fused with the rsqrt(sigma) channel scale

Host does only sharding/layout: batch slicing, weight transpose to
matmul layout (+ f32r pre-round, the device matmul input format), and
s reshape.
"""

import sys
from contextlib import ExitStack

if "/opt/trn_rl_repo" not in sys.path:
    sys.path.insert(0, "/opt/trn_rl_repo")

import numpy as np

import concourse.bacc as bacc
import concourse.mybir as mybir
import concourse.tile as tile
from concourse.bass_utils import run_bass_kernel_spmd

F32 = mybir.dt.float32
F32R = mybir.dt.float32r

N_CORES = 8
B = 16
B2 = B // N_CORES  # samples per core
C = 512
NCH = 4  # 128-partition channel chunks
H = W = 64
EPS = 1e-8
CSCALE = 1.0 / (C * 9) ** 0.5

_NC_CACHE = {}


def _build(psum_bufs: int = 7, yb_inner: bool = True, raw_bufs: int = 3):
    nc = bacc.Bacc("TRN2", target_bir_lowering=False, debug=False)

    x_d = nc.dram_tensor("x", [B2, C, H, W], F32, kind="ExternalInput")
    s_d = nc.dram_tensor("s", [128, NCH, B2], F32, kind="ExternalInput")
    w_d = nc.dram_tensor("w", [128, 9, NCH, C], F32R, kind="ExternalInput")
    o_d = nc.dram_tensor("o", [B2, C, H, W], F32, kind="ExternalOutput")

    with tile.TileContext(nc) as tc, ExitStack() as ctx:
        wpool = ctx.enter_context(tc.tile_pool(name="wpool", bufs=1))
        spool = ctx.enter_context(tc.tile_pool(name="spool", bufs=1))
        sqpool = ctx.enter_context(tc.tile_pool(name="sqpool", bufs=2))
        rawpool = ctx.enter_context(tc.tile_pool(name="rawpool", bufs=raw_bufs))
        xpool = ctx.enter_context(tc.tile_pool(name="xpool", bufs=2))
        opool = ctx.enter_context(tc.tile_pool(name="opool", bufs=4))
        pspool = ctx.enter_context(
            tc.tile_pool(name="pspool", bufs=psum_bufs, space="PSUM")
        )
        sigps = ctx.enter_context(tc.tile_pool(name="sigps", bufs=1, space="PSUM"))

        # ---- weights (pre-rounded f32r), one tile per kpos so conv
        # matmuls can start as soon as the first chunk lands ----
        w_ks = []
        for kpos in range(9):
            wk = wpool.tile([128, NCH, C], F32R, tag=f"w{kpos}", name="wk")
            nc.sync.dma_start(wk[:], w_d[:, kpos])
            w_ks.append(wk)

        s_t = spool.tile([128, NCH, B2], F32)
        nc.sync.dma_start(s_t[:], s_d[:])
        nc.vector.tensor_scalar_mul(s_t[:], s_t[:], CSCALE)
        s2_t = spool.tile([128, NCH, B2], F32)
        nc.vector.tensor_mul(s2_t[:], s_t[:], s_t[:])

        # zeros for pad regions (f32 source for convert-copies)
        z66 = spool.tile([128, 66], F32)
        nc.vector.memset(z66[:], 0.0)

        def prep_half(b, h):
            xts = []
            for ic in range(NCH):
                raw = rawpool.tile([128, 33, 64], F32, tag="raw", name="raw")
                r0 = 0 if h == 0 else 31
                nc.sync.dma_start(
                    raw[:], x_d[b, ic * 128 : (ic + 1) * 128, r0 : r0 + 33, :]
                )
                xt = xpool.tile([128, 34, 66], F32R, tag=f"xt{ic}", name="xt")
                nc.vector.tensor_copy(xt[:, :, 0], z66[:, 0:34])
                nc.vector.tensor_copy(xt[:, :, 65], z66[:, 0:34])
                if h == 0:
                    nc.vector.tensor_copy(xt[:, 0, :], z66[:, 0:66])
                    dst = xt[:, 1:34, 1:65]
                else:
                    nc.vector.tensor_copy(xt[:, 33, :], z66[:, 0:66])
                    dst = xt[:, 0:33, 1:65]
                # scale by s' and round to f32r
                nc.vector.tensor_scalar_mul(dst, raw[:], s_t[:, ic, b : b + 1])
                xts.append(xt)
            return xts

        # first half's x-prep emitted before sigma so DVE serves it first
        xts_00 = prep_half(0, 0)

        # ---- sigma_sq[b, o] = sum_{i,k} w2[i,o] * s'2[i,b] ----
        # q[i, o] = sum_k w2 reduced on DVE; then 16 tiny matmuls
        psig = sigps.tile([128, NCH, B2], F32)
        for ic in range(NCH):
            q = sqpool.tile([128, C], F32, tag="q", name="q")
            sq = sqpool.tile([128, C], F32, tag="sq", name="sq")
            wf = w_ks[0][:].bitcast(F32)
            nc.vector.tensor_mul(q[:], wf[:, ic], wf[:, ic])
            for kpos in range(1, 9):
                wf = w_ks[kpos][:].bitcast(F32)
                nc.vector.tensor_mul(sq[:], wf[:, ic], wf[:, ic])
                nc.vector.tensor_add(q[:], q[:], sq[:])
            for oc in range(NCH):
                # start=True clears the WHOLE bank -> only the global
                # first matmul sets it; later groups overwrite-where-
                # unset via per-element has_written bits.
                nc.tensor.matmul(
                    psig[:, oc, :],
                    q[:, oc * 128 : (oc + 1) * 128],
                    s2_t[:, ic, :],
                    start=(ic == 0 and oc == 0),
                    stop=(ic == 3 and oc == 3),
                    skip_group_check=True,
                )
        sig_t = spool.tile([128, NCH, B2], F32)
        nc.vector.tensor_scalar_add(sig_t[:], psig[:], EPS)
        nc.scalar.sqrt(sig_t[:], sig_t[:])
        nc.vector.reciprocal(sig_t[:], sig_t[:])

        # ---- conv: per sample, 2 halves of 32 output rows ----
        # xt rows 0..33 = zero-padded image rows [h*32, h*32+34)
        for b in range(B2):
            for h in range(2):
                xts = xts_00 if (b == 0 and h == 0) else prep_half(b, h)

                for oc in range(NCH):
                    if yb_inner:
                        accs = [
                            pspool.tile([128, 512], F32, tag="acc", name=f"acc{yy}")
                            for yy in range(4)
                        ]
                        for kpos in range(9):
                            ky, kx = divmod(kpos, 3)
                            for ic in range(NCH):
                                lhsT = w_ks[kpos][:, ic, oc * 128 : (oc + 1) * 128]
                                for yb in range(4):
                                    nc.tensor.matmul(
                                        accs[yb][:],
                                        lhsT,
                                        xts[ic][
                                            :,
                                            yb * 8 + ky : yb * 8 + ky + 8,
                                            kx : kx + 64,
                                        ],
                                        start=(kpos == 0 and ic == 0),
                                        stop=(kpos == 8 and ic == 3),
                                    )
                    else:
                        accs = []
                        for yb in range(4):
                            acc = pspool.tile([128, 512], F32, tag="acc", name="acc")
                            accs.append(acc)
                            for kpos in range(9):
                                ky, kx = divmod(kpos, 3)
                                for ic in range(NCH):
                                    nc.tensor.matmul(
                                        acc[:],
                                        w_ks[kpos][:, ic, oc * 128 : (oc + 1) * 128],
                                        xts[ic][
                                            :,
                                            yb * 8 + ky : yb * 8 + ky + 8,
                                            kx : kx + 64,
                                        ],
                                        start=(kpos == 0 and ic == 0),
                                        stop=(kpos == 8 and ic == 3),
                                    )
                    for yb in range(4):
                        out_t = opool.tile([128, 512], F32, tag="out", name="out")
                        nc.vector.tensor_scalar_mul(out_t[:], accs[yb][:], sig_t[:, oc, b : b + 1])
                        y0 = h * 32 + yb * 8
                        nc.sync.dma_start(
                            o_d[b, oc * 128 : (oc + 1) * 128, y0 : y0 + 8, :],
                            out_t[:],
                        )

    nc.compile()
    return nc


def get_nc(**kwargs):
    key = tuple(sorted(kwargs.items()))
    if key not in _NC_CACHE:
        _NC_CACHE[key] = _build(**kwargs)
    return _NC_CACHE[key]


def _round_f32r(x: np.ndarray) -> np.ndarray:
    """Round fp32 to 11 mantissa bits (RNE) — the f32r matmul format."""
    u = np.ascontiguousarray(x).view(np.uint32)
    low = u & np.uint32(0xFFF)
    half = np.uint32(0x800)
    lsb = (u >> np.uint32(12)) & np.uint32(1)
    rnd = (low > half) | ((low == half) & (lsb == 1))
    out = (u & np.uint32(0xFFFFF000)) + (rnd.astype(np.uint32) << np.uint32(12))
    return out.view(np.float32)


def make_in_maps(x, s, weight):
    """Shard full inputs into 8 per-core input maps."""
    x = np.asarray(x, dtype=np.float32)
    s = np.asarray(s, dtype=np.float32)
    weight = np.asarray(weight, dtype=np.float32)
    w_prep = np.ascontiguousarray(
        weight.reshape(C, NCH, 128, 3, 3).transpose(2, 3, 4, 1, 0).reshape(
            128, 9, NCH, C
        )
    )
    w_prep = _round_f32r(w_prep)
    in_maps = []
    for core in range(N_CORES):
        xs = np.ascontiguousarray(x[core * B2 : (core + 1) * B2])
        ss = np.ascontiguousarray(
            s[core * B2 : (core + 1) * B2].reshape(B2, NCH, 128).transpose(2, 1, 0)
        )
        in_maps.append({"x": xs, "s": ss, "w": w_prep})
    return in_maps


def kernel(x, s, weight):
    nc = get_nc()
    in_maps = make_in_maps(x, s, weight)
    res = run_bass_kernel_spmd(nc, in_maps, list(range(N_CORES)))
    out = np.concatenate([r["o"] for r in res.results], axis=0)
    return out.astype(np.float32)


# revision 8
# speedup vs baseline: 1.1332x; 1.0566x over previous
"""ConvModLayer (StyleGAN2-style modulated 3x3 conv) on 8 Trainium2
NeuronCores — data-parallel over the batch (16 samples -> 2 per core).

Math (equivalent to the reference):
  cscale = 1/sqrt(512*9)
  s' = s * cscale
  sigma_sq[b,o] = sum_{i,ky,kx} (weight[o,i,ky,kx] * s'[b,i])^2
  out[b] = conv3x3(x[b] * s'[b,:,None,None], weight) * rsqrt(sigma_sq[b] + eps)

Device kernel (per core, identical SPMD program):
  - conv done as 9 shifted matmuls accumulated in PSUM over a
    zero-padded 66-wide image layout, operands in float32r
    (fp32 rounded to 11 mantissa bits) for full PE rate
  - sigma_sq via 144 tiny matmuls of squared weights against s'^2
  - PSUM -> SBUF copy fused with the rsqrt(sigma) channel scale

Host does only sharding/layout: batch slicing, weight transpose to
matmul layout (+ f32r pre-round, the device matmul input format), and
s reshape.
"""

import sys
from contextlib import ExitStack

if "/opt/trn_rl_repo" not in sys.path:
    sys.path.insert(0, "/opt/trn_rl_repo")

import numpy as np

import concourse.bacc as bacc
import concourse.mybir as mybir
import concourse.tile as tile
from concourse.bass_utils import run_bass_kernel_spmd

F32 = mybir.dt.float32
F32R = mybir.dt.float32r

N_CORES = 8
B = 16
B2 = B // N_CORES  # samples per core
C = 512
NCH = 4  # 128-partition channel chunks
H = W = 64
EPS = 1e-8
CSCALE = 1.0 / (C * 9) ** 0.5

_NC_CACHE = {}


def _build(psum_bufs: int = 7, yb_inner: bool = True, raw_bufs: int = 3):
    nc = bacc.Bacc("TRN2", target_bir_lowering=False, debug=False)

    x_d = nc.dram_tensor("x", [B2, C, H, W], F32, kind="ExternalInput")
    s_d = nc.dram_tensor("s", [128, NCH, B2], F32, kind="ExternalInput")
    w_d = nc.dram_tensor("w", [128, 9, NCH, C], F32R, kind="ExternalInput")
    o_d = nc.dram_tensor("o", [B2, C, H, W], F32, kind="ExternalOutput")

    with tile.TileContext(nc) as tc, ExitStack() as ctx:
        wpool = ctx.enter_context(tc.tile_pool(name="wpool", bufs=1))
        spool = ctx.enter_context(tc.tile_pool(name="spool", bufs=1))
        sqpool = ctx.enter_context(tc.tile_pool(name="sqpool", bufs=2))
        rawpool = ctx.enter_context(tc.tile_pool(name="rawpool", bufs=raw_bufs))
        xpool = ctx.enter_context(tc.tile_pool(name="xpool", bufs=2))
        opool = ctx.enter_context(tc.tile_pool(name="opool", bufs=4))
        pspool = ctx.enter_context(
            tc.tile_pool(name="pspool", bufs=psum_bufs, space="PSUM")
        )
        sigps = ctx.enter_context(tc.tile_pool(name="sigps", bufs=1, space="PSUM"))

        s_t = spool.tile([128, NCH, B2], F32)
        nc.sync.dma_start(s_t[:], s_d[:])
        nc.vector.tensor_scalar_mul(s_t[:], s_t[:], CSCALE)
        s2_t = spool.tile([128, NCH, B2], F32)
        nc.vector.tensor_mul(s2_t[:], s_t[:], s_t[:])

        # zeros for pad regions (f32 source for convert-copies)
        z66 = spool.tile([128, 66], F32)
        nc.vector.memset(z66[:], 0.0)

        def prep_half(b, h):
            xts = []
            for ic in range(NCH):
                raw = rawpool.tile([128, 33, 64], F32, tag="raw", name="raw")
                r0 = 0 if h == 0 else 31
                nc.sync.dma_start(
                    raw[:], x_d[b, ic * 128 : (ic + 1) * 128, r0 : r0 + 33, :]
                )
                xt = xpool.tile([128, 34, 66], F32R, tag=f"xt{ic}", name="xt")
                nc.vector.tensor_copy(xt[:, :, 0], z66[:, 0:34])
                nc.vector.tensor_copy(xt[:, :, 65], z66[:, 0:34])
                if h == 0:
                    nc.vector.tensor_copy(xt[:, 0, :], z66[:, 0:66])
                    dst = xt[:, 1:34, 1:65]
                else:
                    nc.vector.tensor_copy(xt[:, 33, :], z66[:, 0:66])
                    dst = xt[:, 0:33, 1:65]
                # scale by s' and round to f32r
                nc.vector.tensor_scalar_mul(dst, raw[:], s_t[:, ic, b : b + 1])
                xts.append(xt)
            return xts

        # first half's x-prep emitted FIRST so its DMAs beat the weight
        # stream on the queue and DVE serves its scales first
        xts_00 = prep_half(0, 0)

        # ---- weights (pre-rounded f32r), one tile per kpos so conv
        # matmuls can start as soon as the first chunk lands ----
        w_ks = []
        for kpos in range(9):
            wk = wpool.tile([128, NCH, C], F32R, tag=f"w{kpos}", name="wk")
            nc.sync.dma_start(wk[:], w_d[:, kpos])
            w_ks.append(wk)

        # ---- sigma_sq[b, o] = sum_{i,k} w2[i,o] * s'2[i,b] ----
        # q[i, o] = sum_k w2 reduced on DVE; then 16 tiny matmuls
        psig = sigps.tile([128, NCH, B2], F32)
        for ic in range(NCH):
            q = sqpool.tile([128, C], F32, tag="q", name="q")
            sq = sqpool.tile([128, C], F32, tag="sq", name="sq")
            wf = w_ks[0][:].bitcast(F32)
            nc.vector.tensor_mul(q[:], wf[:, ic], wf[:, ic])
            for kpos in range(1, 9):
                wf = w_ks[kpos][:].bitcast(F32)
                nc.vector.tensor_mul(sq[:], wf[:, ic], wf[:, ic])
                nc.vector.tensor_add(q[:], q[:], sq[:])
            for oc in range(NCH):
                # start=True clears the WHOLE bank -> only the global
                # first matmul sets it; later groups overwrite-where-
                # unset via per-element has_written bits.
                nc.tensor.matmul(
                    psig[:, oc, :],
                    q[:, oc * 128 : (oc + 1) * 128],
                    s2_t[:, ic, :],
                    start=(ic == 0 and oc == 0),
                    stop=(ic == 3 and oc == 3),
                    skip_group_check=True,
                )
        sig_t = spool.tile([128, NCH, B2], F32)
        nc.vector.tensor_scalar_add(sig_t[:], psig[:], EPS)
        nc.scalar.sqrt(sig_t[:], sig_t[:])
        nc.vector.reciprocal(sig_t[:], sig_t[:])

        # ---- conv: per sample, 2 halves of 32 output rows ----
        # xt rows 0..33 = zero-padded image rows [h*32, h*32+34)
        quarters = [(b, h) for b in range(B2) for h in range(2)]
        preps = {0: xts_00}
        for qi, (b, h) in enumerate(quarters):
            xts = preps.pop(qi)
            for oc in range(NCH):
                accs = [
                    pspool.tile([128, 512], F32, tag="acc", name=f"acc{yy}")
                    for yy in range(4)
                ]
                for kpos in range(9):
                    ky, kx = divmod(kpos, 3)
                    for ic in range(NCH):
                        lhsT = w_ks[kpos][:, ic, oc * 128 : (oc + 1) * 128]
                        for yb in range(4):
                            nc.tensor.matmul(
                                accs[yb][:],
                                lhsT,
                                xts[ic][
                                    :,
                                    yb * 8 + ky : yb * 8 + ky + 8,
                                    kx : kx + 64,
                                ],
                                start=(kpos == 0 and ic == 0),
                                stop=(kpos == 8 and ic == 3),
                            )
                if oc == 0 and qi + 1 < len(quarters):
                    # emit next quarter's x-prep ahead of this quarter's
                    # copies so its DMAs/scales get scheduling priority
                    preps[qi + 1] = prep_half(*quarters[qi + 1])
                for yb in range(4):
                    out_t = opool.tile([128, 512], F32, tag="out", name="out")
                    nc.scalar.mul(out_t[:], accs[yb][:], sig_t[:, oc, b : b + 1])
                    y0 = h * 32 + yb * 8
                    nc.sync.dma_start(
                        o_d[b, oc * 128 : (oc + 1) * 128, y0 : y0 + 8, :],
                        out_t[:],
                    )

    nc.compile()
    return nc


def get_nc(**kwargs):
    key = tuple(sorted(kwargs.items()))
    if key not in _NC_CACHE:
        _NC_CACHE[key] = _build(**kwargs)
    return _NC_CACHE[key]


def _round_f32r(x: np.ndarray) -> np.ndarray:
    """Round fp32 to 11 mantissa bits (RNE) — the f32r matmul format."""
    u = np.ascontiguousarray(x).view(np.uint32)
    low = u & np.uint32(0xFFF)
    half = np.uint32(0x800)
    lsb = (u >> np.uint32(12)) & np.uint32(1)
    rnd = (low > half) | ((low == half) & (lsb == 1))
    out = (u & np.uint32(0xFFFFF000)) + (rnd.astype(np.uint32) << np.uint32(12))
    return out.view(np.float32)


def make_in_maps(x, s, weight):
    """Shard full inputs into 8 per-core input maps."""
    x = np.asarray(x, dtype=np.float32)
    s = np.asarray(s, dtype=np.float32)
    weight = np.asarray(weight, dtype=np.float32)
    w_prep = np.ascontiguousarray(
        weight.reshape(C, NCH, 128, 3, 3).transpose(2, 3, 4, 1, 0).reshape(
            128, 9, NCH, C
        )
    )
    w_prep = _round_f32r(w_prep)
    in_maps = []
    for core in range(N_CORES):
        xs = np.ascontiguousarray(x[core * B2 : (core + 1) * B2])
        ss = np.ascontiguousarray(
            s[core * B2 : (core + 1) * B2].reshape(B2, NCH, 128).transpose(2, 1, 0)
        )
        in_maps.append({"x": xs, "s": ss, "w": w_prep})
    return in_maps


def kernel(x, s, weight):
    nc = get_nc()
    in_maps = make_in_maps(x, s, weight)
    res = run_bass_kernel_spmd(nc, in_maps, list(range(N_CORES)))
    out = np.concatenate([r["o"] for r in res.results], axis=0)
    return out.astype(np.float32)


# revision 9
# speedup vs baseline: 1.1431x; 1.0088x over previous
"""ConvModLayer (StyleGAN2-style modulated 3x3 conv) on 8 Trainium2
NeuronCores — data-parallel over the batch (16 samples -> 2 per core).

Math (equivalent to the reference):
  cscale = 1/sqrt(512*9)
  s' = s * cscale
  sigma_sq[b,o] = sum_{i,ky,kx} (weight[o,i,ky,kx] * s'[b,i])^2
  out[b] = conv3x3(x[b] * s'[b,:,None,None], weight) * rsqrt(sigma_sq[b] + eps)

Device kernel (per core, identical SPMD program):
  - conv done as 9 shifted matmuls accumulated in PSUM over a
    zero-padded 66-wide image layout, operands in float32r
    (fp32 rounded to 11 mantissa bits) for full PE rate
  - sigma_sq via 144 tiny matmuls of squared weights against s'^2
  - PSUM -> SBUF copy fused with the rsqrt(sigma) channel scale

Host does only sharding/layout: batch slicing, weight transpose to
matmul layout (+ f32r pre-round, the device matmul input format), and
s reshape.
"""

import sys
from contextlib import ExitStack

if "/opt/trn_rl_repo" not in sys.path:
    sys.path.insert(0, "/opt/trn_rl_repo")

import numpy as np

import concourse.bacc as bacc
import concourse.mybir as mybir
import concourse.tile as tile
from concourse.bass_utils import run_bass_kernel_spmd

F32 = mybir.dt.float32
F32R = mybir.dt.float32r

N_CORES = 8
B = 16
B2 = B // N_CORES  # samples per core
C = 512
NCH = 4  # 128-partition channel chunks
H = W = 64
EPS = 1e-8
CSCALE = 1.0 / (C * 9) ** 0.5

_NC_CACHE = {}


def _build(psum_bufs: int = 7, yb_inner: bool = True, raw_bufs: int = 3):
    nc = bacc.Bacc("TRN2", target_bir_lowering=False, debug=False)

    x_d = nc.dram_tensor("x", [B2, C, H, W], F32, kind="ExternalInput")
    s_d = nc.dram_tensor("s", [128, NCH, B2], F32, kind="ExternalInput")
    w_d = nc.dram_tensor("w", [128, 9, NCH, C], F32R, kind="ExternalInput")
    o_d = nc.dram_tensor("o", [B2, C, H, W], F32, kind="ExternalOutput")

    with tile.TileContext(nc) as tc, ExitStack() as ctx:
        wpool = ctx.enter_context(tc.tile_pool(name="wpool", bufs=1))
        spool = ctx.enter_context(tc.tile_pool(name="spool", bufs=1))
        sqpool = ctx.enter_context(tc.tile_pool(name="sqpool", bufs=2))
        rawpool = ctx.enter_context(tc.tile_pool(name="rawpool", bufs=raw_bufs))
        xpool = ctx.enter_context(tc.tile_pool(name="xpool", bufs=2))
        opool = ctx.enter_context(tc.tile_pool(name="opool", bufs=4))
        pspool = ctx.enter_context(
            tc.tile_pool(name="pspool", bufs=psum_bufs, space="PSUM")
        )
        sigps = ctx.enter_context(tc.tile_pool(name="sigps", bufs=1, space="PSUM"))

        s_t = spool.tile([128, NCH, B2], F32)
        nc.sync.dma_start(s_t[:], s_d[:])
        nc.vector.tensor_scalar_mul(s_t[:], s_t[:], CSCALE)
        s2_t = spool.tile([128, NCH, B2], F32)
        nc.vector.tensor_mul(s2_t[:], s_t[:], s_t[:])

        # zeros for pad regions (f32 source for convert-copies)
        z66 = spool.tile([128, 66], F32)
        nc.vector.memset(z66[:], 0.0)

        def prep_half(b, h, ics=tuple(range(NCH))):
            xts = []
            for ic in ics:
                raw = rawpool.tile([128, 33, 64], F32, tag="raw", name="raw")
                r0 = 0 if h == 0 else 31
                nc.sync.dma_start(
                    raw[:], x_d[b, ic * 128 : (ic + 1) * 128, r0 : r0 + 33, :]
                )
                xt = xpool.tile([128, 34, 66], F32R, tag=f"xt{ic}", name="xt")
                nc.vector.tensor_copy(xt[:, :, 0], z66[:, 0:34])
                nc.vector.tensor_copy(xt[:, :, 65], z66[:, 0:34])
                if h == 0:
                    nc.vector.tensor_copy(xt[:, 0, :], z66[:, 0:66])
                    dst = xt[:, 1:34, 1:65]
                else:
                    nc.vector.tensor_copy(xt[:, 33, :], z66[:, 0:66])
                    dst = xt[:, 0:33, 1:65]
                # scale by s' and round to f32r
                nc.vector.tensor_scalar_mul(dst, raw[:], s_t[:, ic, b : b + 1])
                xts.append(xt)
            return xts

        # DMA emission order = arrival order on the single hw queue, so
        # interleave: x chunk 0 -> w[0] -> x chunks 1..3 -> w[1..8].
        # First conv matmul needs only xt[0] + w[0] (~2.1 MB instead of
        # ~13 MB of queue traffic).
        w_ks = []

        def emit_w(kpos):
            wk = wpool.tile([128, NCH, C], F32R, tag=f"w{kpos}", name="wk")
            nc.sync.dma_start(wk[:], w_d[:, kpos])
            w_ks.append(wk)

        xts_00 = prep_half(0, 0, ics=(0,))
        emit_w(0)
        xts_00 += prep_half(0, 0, ics=(1, 2, 3))
        for kpos in range(1, 9):
            emit_w(kpos)

        # ---- sigma_sq[b, o] = sum_{i,k} w2[i,o] * s'2[i,b] ----
        # q[i, o] = sum_k w2 reduced on DVE; then 16 tiny matmuls
        psig = sigps.tile([128, NCH, B2], F32)
        for ic in range(NCH):
            q = sqpool.tile([128, C], F32, tag="q", name="q")
            sq = sqpool.tile([128, C], F32, tag="sq", name="sq")
            wf = w_ks[0][:].bitcast(F32)
            nc.vector.tensor_mul(q[:], wf[:, ic], wf[:, ic])
            for kpos in range(1, 9):
                wf = w_ks[kpos][:].bitcast(F32)
                nc.vector.tensor_mul(sq[:], wf[:, ic], wf[:, ic])
                nc.vector.tensor_add(q[:], q[:], sq[:])
            for oc in range(NCH):
                # start=True clears the WHOLE bank -> only the global
                # first matmul sets it; later groups overwrite-where-
                # unset via per-element has_written bits.
                nc.tensor.matmul(
                    psig[:, oc, :],
                    q[:, oc * 128 : (oc + 1) * 128],
                    s2_t[:, ic, :],
                    start=(ic == 0 and oc == 0),
                    stop=(ic == 3 and oc == 3),
                    skip_group_check=True,
                )
        sig_t = spool.tile([128, NCH, B2], F32)
        nc.vector.tensor_scalar_add(sig_t[:], psig[:], EPS)
        nc.scalar.sqrt(sig_t[:], sig_t[:])
        nc.vector.reciprocal(sig_t[:], sig_t[:])

        # ---- conv: per sample, 2 halves of 32 output rows ----
        # xt rows 0..33 = zero-padded image rows [h*32, h*32+34)
        quarters = [(b, h) for b in range(B2) for h in range(2)]
        preps = {0: xts_00}
        for qi, (b, h) in enumerate(quarters):
            xts = preps.pop(qi)
            for oc in range(NCH):
                accs = [
                    pspool.tile([128, 512], F32, tag="acc", name=f"acc{yy}")
                    for yy in range(4)
                ]
                for kpos in range(9):
                    ky, kx = divmod(kpos, 3)
                    for ic in range(NCH):
                        lhsT = w_ks[kpos][:, ic, oc * 128 : (oc + 1) * 128]
                        for yb in range(4):
                            nc.tensor.matmul(
                                accs[yb][:],
                                lhsT,
                                xts[ic][
                                    :,
                                    yb * 8 + ky : yb * 8 + ky + 8,
                                    kx : kx + 64,
                                ],
                                start=(kpos == 0 and ic == 0),
                                stop=(kpos == 8 and ic == 3),
                            )
                if oc == 0 and qi + 1 < len(quarters):
                    # emit next quarter's x-prep ahead of this quarter's
                    # copies so its DMAs/scales get scheduling priority
                    preps[qi + 1] = prep_half(*quarters[qi + 1])
                last = qi == len(quarters) - 1 and oc == NCH - 1
                for yb in range(4):
                    out_t = opool.tile([128, 512], F32, tag="out", name="out")
                    if last:
                        # DVE is idle here and ~3x faster than ACT per
                        # copy — shortens the end-of-kernel drain chain
                        nc.vector.tensor_scalar_mul(
                            out_t[:], accs[yb][:], sig_t[:, oc, b : b + 1]
                        )
                    else:
                        nc.scalar.mul(out_t[:], accs[yb][:], sig_t[:, oc, b : b + 1])
                    y0 = h * 32 + yb * 8
                    nc.sync.dma_start(
                        o_d[b, oc * 128 : (oc + 1) * 128, y0 : y0 + 8, :],
                        out_t[:],
                    )

    nc.compile()
    return nc


def get_nc(**kwargs):
    key = tuple(sorted(kwargs.items()))
    if key not in _NC_CACHE:
        _NC_CACHE[key] = _build(**kwargs)
    return _NC_CACHE[key]


def _round_f32r(x: np.ndarray) -> np.ndarray:
    """Round fp32 to 11 mantissa bits (RNE) — the f32r matmul format."""
    u = np.ascontiguousarray(x).view(np.uint32)
    low = u & np.uint32(0xFFF)
    half = np.uint32(0x800)
    lsb = (u >> np.uint32(12)) & np.uint32(1)
    rnd = (low > half) | ((low == half) & (lsb == 1))
    out = (u & np.uint32(0xFFFFF000)) + (rnd.astype(np.uint32) << np.uint32(12))
    return out.view(np.float32)


def make_in_maps(x, s, weight):
    """Shard full inputs into 8 per-core input maps."""
    x = np.asarray(x, dtype=np.float32)
    s = np.asarray(s, dtype=np.float32)
    weight = np.asarray(weight, dtype=np.float32)
    w_prep = np.ascontiguousarray(
        weight.reshape(C, NCH, 128, 3, 3).transpose(2, 3, 4, 1, 0).reshape(
            128, 9, NCH, C
        )
    )
    w_prep = _round_f32r(w_prep)
    in_maps = []
    for core in range(N_CORES):
        xs = np.ascontiguousarray(x[core * B2 : (core + 1) * B2])
        ss = np.ascontiguousarray(
            s[core * B2 : (core + 1) * B2].reshape(B2, NCH, 128).transpose(2, 1, 0)
        )
        in_maps.append({"x": xs, "s": ss, "w": w_prep})
    return in_maps


def kernel(x, s, weight):
    nc = get_nc()
    in_maps = make_in_maps(x, s, weight)
    res = run_bass_kernel_spmd(nc, in_maps, list(range(N_CORES)))
    out = np.concatenate([r["o"] for r in res.results], axis=0)
    return out.astype(np.float32)
